# revision 14
# baseline (speedup 1.0000x reference)
"""Trainium2 Bass kernel for a dense transformer decoder block.

Strategy (8 NeuronCores):
  - Attention tensor-parallel over heads (2 heads/core); activations in
    transposed layout [D, tokens]; all matmuls bf16 with fp32 PSUM.
  - Instead of ReduceScattering the wo *outputs* (16.8 MB), the per-head
    attention outputs are exchanged with chunked AllToAlls (2 MB total,
    bf16, DeepSpeed-Ulysses style): after the A2A each core holds all 16
    heads for its own 512 tokens and computes the full wo locally (same
    FLOPs, overlapped chunk-by-chunk with the remaining attention), adds
    the exact f32 x residual from a per-core xres input, and keeps h in
    SBUF. No reduction collective at all.
  - FFN is data-parallel: each core runs the FULL FFN on its 512 tokens,
    streaming the full w1/w2 (bf16) from HBM under the matmuls. rmsnorm
    scaling is deferred through relu and the (linear) down-projection
    (relu(r*u) = r*relu(u), r>0), applied once on the 16 down outputs,
    so the PE never waits on the stats chain. Each core's (h + down) IS
    the final output for its tokens.
  - Engine balance: squares on DVE, exp on ACT, reciprocals via the fast
    DVE approximation, masks added via PE accumulation; stats computed
    one q-tile ahead so the PE runs dense.
  - Causality is not hardcoded: the mask input is classified host-side
    into skip / plain / mixed 128x512 blocks; mixed tiles are shipped
    as constants (4 distinct tiles for a causal mask).
"""

import os
import sys

try:  # the axon sitecustomize usually provides concourse already
    import concourse.bass  # noqa: F401
except ImportError:  # pragma: no cover
    sys.path.insert(0, "/opt/trn_rl_repo")

from contextlib import ExitStack

import ml_dtypes
import numpy as np

import concourse.bacc as bacc
import concourse.tile as tile
from concourse import mybir
from concourse.bass_utils import run_bass_kernel_spmd
from concourse.masks import make_identity

F32 = mybir.dt.float32
BF16 = mybir.dt.bfloat16
F16 = mybir.dt.float16
N_CORES = 8
P = 128
QW = 512  # q-tile / token-tile width
EPS = 1e-6
AF = mybir.ActivationFunctionType
ALU = mybir.AluOpType
BF16_NP = ml_dtypes.bfloat16
G = 4  # A2A chunks (one per pair of q-tiles)
SL = 128  # tokens per core-slice per chunk


def ts(i, w):
    return slice(i * w, (i + 1) * w)


def _classify_mask(mask, S):
    """mask: [S, S] additive (q, k). Returns (table, tiles).
    table[(kt, j)] = 'skip' | 'plain' | int mask-tile index.
    tiles: list of [128, QW] float32 arrays in scoresT ([k, q]) layout."""
    table = {}
    tiles = []
    keys = {}
    for j in range(S // QW):
        for kt in range(S // P):
            sub = mask[ts(j, QW), ts(kt, P)]  # [q, k]
            if np.all(sub <= -1e8):
                table[(kt, j)] = "skip"
            elif np.all(sub == 0.0):
                table[(kt, j)] = "plain"
            else:
                t = np.ascontiguousarray(sub.T.astype(np.float32))  # [k, q]
                key = t.tobytes()
                if key not in keys:
                    keys[key] = len(tiles)
                    tiles.append(t)
                table[(kt, j)] = keys[key]
    return table, tiles


def build_program(B, S, D, H, HID, mask_table, n_mask):
    HD = 128
    assert D == (D // P) * P and H * HD == D
    HPC = H // N_CORES            # heads per core
    assert HPC * N_CORES == H
    C = D // P                    # contraction chunks over D
    S_TILES = S // QW             # q tiles per batch
    KT = S // P                   # k tiles per batch
    HIDC = HID // P               # hidden tiles total (64)
    W1CH = 16                     # w1 stream chunks
    HTPC = HIDC // W1CH           # hid tiles per w1 chunk (4)

    nc = bacc.Bacc(trn_type="TRN2", num_devices=N_CORES)

    xbh = nc.dram_tensor(
        "xbh", [B, S // QW, P, D // P, QW], BF16, kind="ExternalInput"
    ).ap()
    xres = nc.dram_tensor("xres", [G, P, C, SL], F32, kind="ExternalInput").ap()
    wq = nc.dram_tensor("wq", [C, P, HPC * HD], BF16, kind="ExternalInput").ap()
    wk = nc.dram_tensor("wk", [C, P, HPC * HD], BF16, kind="ExternalInput").ap()
    wv = nc.dram_tensor("wv", [C, P, HPC * HD], BF16, kind="ExternalInput").ap()
    woh = nc.dram_tensor("woh", [P, H, D], BF16, kind="ExternalInput").ap()
    wo3h = nc.dram_tensor("wo3h", [C, P, H, P], BF16, kind="ExternalInput").ap()
    w1h = nc.dram_tensor("w1h", [W1CH, P, C, QW], BF16, kind="ExternalInput").ap()
    w2h = nc.dram_tensor("w2h", [C, P, HIDC, P], BF16, kind="ExternalInput").ap()
    mk = None
    if n_mask:
        mk = nc.dram_tensor("mk", [n_mask, P, QW], BF16, kind="ExternalInput").ap()

    a2a_in = [
        nc.dram_tensor(f"a2a_in{g}", [N_CORES, HPC, P, SL], BF16)
        for g in range(G)
    ]
    a2a_out = [
        nc.dram_tensor(f"a2a_out{g}", [N_CORES, HPC, P, SL], BF16)
        for g in range(G)
    ]
    out = nc.dram_tensor("out", [C, P, QW], F32, kind="ExternalOutput").ap()

    groups = [list(range(N_CORES))]

    with tile.TileContext(nc) as tc, ExitStack() as ctx:
        const = ctx.enter_context(tc.tile_pool(name="const", bufs=1))
        stats = ctx.enter_context(tc.tile_pool(name="stats", bufs=2))
        sqp = ctx.enter_context(tc.tile_pool(name="sq", bufs=2))
        hfp = ctx.enter_context(tc.tile_pool(name="hf", bufs=1))
        xrp = ctx.enter_context(tc.tile_pool(name="xr", bufs=1))
        aop = ctx.enter_context(tc.tile_pool(name="ao", bufs=2))
        psum = ctx.enter_context(tc.tile_pool(name="psum", bufs=1, space="PSUM"))

        ones_f32 = const.tile([P, P], F32)
        nc.vector.memset(ones_f32[:], 1.0)
        ones = const.tile([P, P], BF16)
        nc.vector.tensor_copy(ones[:], ones_f32[:])
        eps_p1 = const.tile([P, 1], F32)
        nc.vector.memset(eps_p1[:], EPS)
        ident = const.tile([P, P], F32)
        make_identity(nc, ident[:])
        ident_b = const.tile([P, P], BF16)
        nc.vector.tensor_copy(ident_b[:], ident[:])

        # h for this core's 512 tokens, assembled chunk by chunk
        hf = hfp.tile([P, C, QW], F16, tag="hf")

        # ---------------- attention phase ----------------
        with tc.tile_pool(name="wqkv", bufs=1) as wqkvp, \
             tc.tile_pool(name="xa", bufs=2) as xap, \
             tc.tile_pool(name="qkv", bufs=1) as qkvp, \
             tc.tile_pool(name="exp", bufs=3) as expp, \
             tc.tile_pool(name="attn", bufs=1) as attp:
            wq_sb = wqkvp.tile([P, C, HPC * HD], BF16, tag="wq")
            nc.sync.dma_start(wq_sb[:], wq.rearrange("c p o -> p c o"))
            wk_sb = wqkvp.tile([P, C, HPC * HD], BF16, tag="wk")
            nc.sync.dma_start(wk_sb[:], wk.rearrange("c p o -> p c o"))
            wv_sb = wqkvp.tile([P, C, HPC * HD], BF16, tag="wv")
            nc.scalar.dma_start(wv_sb[:], wv.rearrange("c p o -> p c o"))
            mtiles = None
            if n_mask:
                mtiles = wqkvp.tile([P, n_mask, QW], BF16, tag="mk")
                nc.scalar.dma_start(mtiles[:], mk.rearrange("n p q -> p n q"))
            wo_sb = wqkvp.tile([P, H, D], BF16, tag="wo")

            def emit_a2a(g):
                nc.gpsimd.collective_compute(
                    "AllToAll",
                    ALU.bypass,
                    replica_groups=groups,
                    ins=[a2a_in[g].ap().opt()],
                    outs=[a2a_out[g].ap().opt()],
                )

            def stats_mm(xb):
                """sq on DVE, column sums via PE."""
                cs = psum.tile([P, QW], F32, tag="mm", bufs=3)
                for c in range(C):
                    sq = sqp.tile([P, QW], BF16, tag="sq")
                    nc.vector.tensor_mul(sq[:], xb[:, c, :], xb[:, c, :])
                    nc.tensor.matmul(
                        cs[:], ones[:], sq[:], start=(c == 0), stop=(c == C - 1)
                    )
                return cs

            def stats_fin(cs):
                """sqrt on ACT (emitted outside the exp burst) + fast recip."""
                rms = stats.tile([P, QW], F32, tag="rms")
                nc.scalar.activation(
                    rms[:], cs[:], AF.Sqrt, bias=eps_p1[:], scale=1.0 / D
                )
                rinv = stats.tile([P, QW], F32, tag="rinv")
                nc.vector.reciprocal_approx_fast(rinv[:], rms[:])
                return rinv

            def emit_wo(g):
                """Local full-wo for this core's chunk-g tokens + x residual
                -> hf columns [g*SL, (g+1)*SL)."""
                ao = aop.tile([P, H, SL], BF16, tag="ao")
                nc.sync.dma_start(
                    ao[:], a2a_out[g].ap().rearrange("s h p t -> p (s h) t")
                )
                xr = xrp.tile([P, C, SL], F32, tag="xr")
                nc.gpsimd.dma_start(xr[:], xres[g])
                for ot in range(C):
                    po = psum.tile([P, SL], F32, tag="mm", bufs=3)
                    for oc in range(H):
                        nc.tensor.matmul(
                            po[:],
                            wo_sb[:, oc, ts(ot, P)],
                            ao[:, oc, :],
                            start=(oc == 0),
                            stop=(oc == H - 1),
                        )
                    nc.vector.tensor_add(
                        hf[:, ot, ts(g, SL)], xr[:, ot, :], po[:]
                    )

            for b in range(B):
                # prefetch x tiles of this batch (host-precast bf16; the
                # DMA hardware defers transfers until the WAR deps clear)
                xbs = []
                for j in range(S_TILES):
                    xb = xap.tile([P, C, QW], BF16, tag="xb", name=f"xb{b}_{j}")
                    nc.sync.dma_start(xb[:], xbh[b, j])
                    xbs.append(xb)

                rinv_next = stats_fin(stats_mm(xbs[0]))

                kT = qkvp.tile([P, HPC, S], BF16, tag="kT")
                vN = qkvp.tile([P, KT, HPC * HD], BF16, tag="vN")
                for j in range(S_TILES):
                    g = 2 * b + j // 2
                    r = j % 2
                    xb = xbs[j]
                    rinv = rinv_next
                    # q/k projections from RAW x; rinv folded in at eviction
                    qTs = qkvp.tile([P, HPC, QW], BF16, tag="qT", bufs=2)
                    for h in range(HPC):
                        for w_sb, dst in ((wq_sb, qTs), (wk_sb, kT)):
                            pp = psum.tile([P, QW], F32, tag="mm", bufs=3)
                            for c in range(C):
                                nc.tensor.matmul(
                                    pp[:],
                                    w_sb[:, c, ts(h, HD)],
                                    xb[:, c, :],
                                    start=(c == 0),
                                    stop=(c == C - 1),
                                )
                            if dst is qTs:
                                nc.vector.tensor_mul(qTs[:, h, :], pp[:], rinv[:])
                            else:
                                nc.vector.tensor_mul(
                                    kT[:, h, ts(j, QW)], pp[:], rinv[:]
                                )
                    # rinv transposed to token-partition layout for v scaling
                    rcol = stats.tile([P, QW // P], F32, tag="rcol")
                    for sub in range(QW // P):
                        tp = psum.tile([P, P], F32, tag="mm", bufs=3)
                        nc.tensor.transpose(tp[:], rinv[:, ts(sub, P)], ident[:])
                        nc.vector.tensor_copy(rcol[:, sub : sub + 1], tp[:, 0:1])
                    # v in natural layout; rinv via per-partition scalar
                    for sub in range(QW // P):
                        pv = psum.tile([P, QW], F32, tag="mm", bufs=3)
                        for c in range(C):
                            nc.tensor.matmul(
                                pv[:, : HPC * HD],
                                xb[:, c, ts(sub, P)],
                                wv_sb[:, c, :],
                                start=(c == 0),
                                stop=(c == C - 1),
                            )
                        nc.vector.tensor_scalar_mul(
                            vN[:, j * (QW // P) + sub, :],
                            pv[:, : HPC * HD],
                            rcol[:, sub : sub + 1],
                        )

                    # stats matmuls for the NEXT q-tile (ACT sqrt emitted
                    # after the exp burst below)
                    cs_next = stats_mm(xbs[j + 1]) if j + 1 < S_TILES else None

                    # -------- attention for q-tile j --------
                    attnT = attp.tile([P, HPC, QW], BF16, tag="attnT", bufs=2)
                    for h in range(HPC):
                        kts = [
                            kt for kt in range(KT) if mask_table[(kt, j)] != "skip"
                        ]
                        pa = psum.tile([P, QW], F32, tag="pv", bufs=1)
                        den = psum.tile([P, QW], F32, tag="stat", bufs=1)
                        n_k = len(kts)
                        exs = [None] * n_k

                        def _den_pv(i):
                            kt = kts[i]
                            nc.tensor.matmul(
                                den[:], ones[:], exs[i][:],
                                start=(i == 0), stop=(i == n_k - 1),
                            )
                            nc.tensor.matmul(
                                pa[:],
                                vN[:, kt, ts(h, HD)],
                                exs[i][:],
                                start=(i == 0),
                                stop=(i == n_k - 1),
                            )

                        for i, kt in enumerate(kts):
                            msc = psum.tile([P, QW], F32, tag="score", bufs=3)
                            mt = mask_table[(kt, j)]
                            if mt != "plain":
                                # additive mask via PE accumulation: PSUM
                                # starts as the mask, score accumulates on top
                                nc.tensor.matmul(
                                    msc[:], ident_b[:], mtiles[:, mt, :],
                                    start=True, stop=False,
                                )
                            nc.tensor.matmul(
                                msc[:],
                                kT[:, h, ts(kt, P)],
                                qTs[:, h, :],
                                start=(mt == "plain"),
                                stop=True,
                            )
                            ex = expp.tile([P, QW], BF16, tag="exp")
                            nc.scalar.activation(ex[:], msc[:], AF.Exp)
                            exs[i] = ex
                            if i > 0:
                                _den_pv(i - 1)
                        _den_pv(n_k - 1)
                        rec = stats.tile([P, QW], F32, tag="rec")
                        nc.vector.reciprocal_approx_fast(rec[:], den[:])
                        nc.vector.tensor_mul(attnT[:, h, :], pa[:], rec[:])

                    if cs_next is not None:
                        rinv_next = stats_fin(cs_next)

                    if r == 0 and g > 0:
                        # local wo for the PREVIOUS chunk (its A2A completed
                        # during this q-tile's attention)
                        emit_wo(g - 1)

                    # ---- ship attnT head-blocks into the A2A input ----
                    for h in range(HPC):
                        nc.sync.dma_start(
                            a2a_in[g].ap()[4 * r : 4 * r + 4, h, :, :].rearrange(
                                "s p t -> p s t"
                            ),
                            attnT[:, h, :],
                        )
                    if r == 1:
                        emit_a2a(g)
                        if g == 0:
                            # deferred big weight load: the gpsimd queue just
                            # unblocked at A2A-0 completion, so this transfer
                            # does not compete with the startup-critical DMAs
                            nc.gpsimd.dma_start(
                                wo_sb[:, :, : D // 2], woh[:, :, : D // 2]
                            )
                            nc.gpsimd.dma_start(
                                wo_sb[:, :, D // 2 :], woh[:, :, D // 2 :]
                            )
        # ---------------- FFN phase (data-parallel, 512 tokens/core) --------
        with tc.tile_pool(name="w1p", bufs=2) as w1p, \
             tc.tile_pool(name="w2p", bufs=2) as w2p, \
             tc.tile_pool(name="wo3p", bufs=2) as wo3p, \
             tc.tile_pool(name="up", bufs=1) as upp, \
             tc.tile_pool(name="oev", bufs=3) as oevp:
            up_sb = upp.tile([P, HIDC, QW], BF16, tag="up")
            HQ = QW // 2

            def up_pass(half):
                """up-projection for one half of the token columns; w1 is
                streamed (again) for each half. relu needs NO rinv (deferred
                through the down projection), so this runs with no stats dep:
                half A fills the A2A-3 + wo(3) tail with dense PE work."""
                cols = ts(half, HQ)
                for ch in range(W1CH):
                    w1c = w1p.tile([P, C, QW], BF16, tag="w1", name=f"w1{half}_{ch}")
                    nc.sync.dma_start(w1c[:], w1h[ch])
                    for hti in range(HTPC):
                        pu = psum.tile([P, HQ], F32, tag="mm", bufs=3)
                        for c in range(C):
                            nc.tensor.matmul(
                                pu[:],
                                w1c[:, c, ts(hti, P)],
                                hf[:, c, cols],
                                start=(c == 0),
                                stop=(c == C - 1),
                            )
                        nc.scalar.activation(
                            up_sb[:, ch * HTPC + hti, cols], pu[:], AF.Relu
                        )

            # ---- half A (token cols 0:256 = chunks 0,1) fills the tail ----
            up_pass(0)

            # ---- wo for the last chunk, with re-streamed wo weights ----
            ao = aop.tile([P, H, SL], BF16, tag="ao", name="ao3")
            nc.sync.dma_start(
                ao[:], a2a_out[G - 1].ap().rearrange("s h p t -> p (s h) t")
            )
            xr = xrp.tile([P, C, SL], F32, tag="xr", name="xr3")
            nc.sync.dma_start(xr[:], xres[G - 1])
            for ot in range(C):
                wo3 = wo3p.tile([P, H, P], BF16, tag="wo3")
                nc.sync.dma_start(wo3[:], wo3h[ot])
                po = psum.tile([P, SL], F32, tag="mm", bufs=3)
                for oc in range(H):
                    nc.tensor.matmul(
                        po[:],
                        wo3[:, oc, :],
                        ao[:, oc, :],
                        start=(oc == 0),
                        stop=(oc == H - 1),
                    )
                nc.vector.tensor_add(
                    hf[:, ot, ts(G - 1, SL)], xr[:, ot, :], po[:]
                )

            # rmsnorm2 stats (consumed only at the down outputs)
            cs = psum.tile([P, QW], F32, tag="mm", bufs=3)
            for c in range(C):
                sq = sqp.tile([P, QW], BF16, tag="sq")
                nc.vector.tensor_mul(sq[:], hf[:, c, :], hf[:, c, :])
                nc.tensor.matmul(
                    cs[:], ones[:], sq[:], start=(c == 0), stop=(c == C - 1)
                )
            rms2 = stats.tile([P, QW], F32, tag="rms")
            nc.scalar.activation(
                rms2[:], cs[:], AF.Sqrt, bias=eps_p1[:], scale=1.0 / D
            )
            r2 = stats.tile([P, QW], F32, tag="rinv")
            nc.vector.reciprocal_approx_fast(r2[:], rms2[:])

            # ---- half B (token cols 256:512) ----
            up_pass(1)

            for ot in range(C):
                w2c = w2p.tile([P, HIDC, P], BF16, tag="w2")
                nc.sync.dma_start(w2c[:], w2h[ot])
                pd = psum.tile([P, QW], F32, tag="mm", bufs=3)
                for hc in range(HIDC):
                    nc.tensor.matmul(
                        pd[:],
                        w2c[:, hc, :],
                        up_sb[:, hc, :],
                        start=(hc == 0),
                        stop=(hc == HIDC - 1),
                    )
                dn = oevp.tile([P, QW], F32, tag="dn")
                nc.vector.tensor_mul(dn[:], pd[:], r2[:])
                oev = oevp.tile([P, QW], F32, tag="oev")
                nc.vector.tensor_add(oev[:], hf[:, ot, :], dn[:])
                nc.sync.dma_start(out[ot], oev[:])

    nc.compile()
    return nc


_CACHE = {}
LAST_RESULT = None


def _get_program(B, S, D, H, HID, mask_table, n_mask, mask_key):
    key = (B, S, D, H, HID, mask_key)
    if key not in _CACHE:
        _CACHE[key] = build_program(B, S, D, H, HID, mask_table, n_mask)
    return _CACHE[key]


def _core_tokens(core, S):
    """Global token start for (core, chunk g) blocks of SL tokens."""
    toks = []
    for g in range(G):
        b = g // 2
        j = 2 * (g % 2) + core // 4
        toks.append(S * b + QW * j + SL * (core % 4))
    return toks


def kernel(x, mask, wq, wk, wv, wo, w1, w2, attn_norm_w, ffn_norm_w):
    x = np.asarray(x, dtype=np.float32)
    mask = np.asarray(mask, dtype=np.float32)
    wq, wk, wv, wo = (np.asarray(a, dtype=np.float32) for a in (wq, wk, wv, wo))
    w1, w2 = np.asarray(w1, dtype=np.float32), np.asarray(w2, dtype=np.float32)
    attn_norm_w = np.asarray(attn_norm_w, dtype=np.float32)
    ffn_norm_w = np.asarray(ffn_norm_w, dtype=np.float32)

    B, S, D = x.shape
    H = D // 128  # HD is fixed at 128 (= SBUF partition count)
    HID = w1.shape[0]
    HD = D // H
    HPC = H // N_CORES
    C = D // P
    HIDC = HID // P
    W1CH = 16

    mask_table, mtiles_np = _classify_mask(
        np.broadcast_to(mask, (1, 1, S, S))[0, 0], S
    )
    mask_key = hash(tuple(sorted((k, str(v)) for k, v in mask_table.items())))
    nc = _get_program(B, S, D, H, HID, mask_table, len(mtiles_np), mask_key)

    # ---- host-side prep ----
    # xbh[b, j, p, c, t] = x[b, j*QW + t, c*128 + p] in bf16
    xbh = np.ascontiguousarray(
        x.reshape(B, S // QW, QW, C, P).transpose(0, 1, 4, 3, 2)
    ).astype(BF16_NP)
    wq_f = (wq * attn_norm_w[None, :]) / np.sqrt(HD)
    wk_f = wk * attn_norm_w[None, :]
    wv_f = wv * attn_norm_w[None, :]
    w1_f = w1 * ffn_norm_w[None, :]

    # full weights, replicated on every core
    # woh[p, oc, o] = wo[o, oc*128 + p]
    wohost = np.ascontiguousarray(
        wo.T.reshape(H, P, D).transpose(1, 0, 2)
    ).astype(BF16_NP)
    # wo3h[ot, p, oc, o] = wo[ot*128 + o, oc*128 + p]
    wo3host = np.ascontiguousarray(
        wo.reshape(C, P, H, P).transpose(0, 3, 2, 1)
    ).astype(BF16_NP)
    # w1h[ch, p, c, o] = w1_f[hid = ch*512 + o, d = c*128 + p]
    w1host = np.ascontiguousarray(
        w1_f.reshape(W1CH, QW, C, P).transpose(0, 3, 2, 1)
    ).astype(BF16_NP)
    # w2h[ot, p, hc, o] = w2[d_out = ot*128 + o, hid = hc*128 + p]
    w2host = np.ascontiguousarray(
        w2.reshape(C, P, HIDC, P).transpose(0, 3, 2, 1)
    ).astype(BF16_NP)

    xf = x.reshape(B * S, D)
    in_maps = []
    for c in range(N_CORES):
        hs = slice(c * HPC * HD, (c + 1) * HPC * HD)
        qs = np.ascontiguousarray(wq_f[hs].T).reshape(C, P, HPC * HD).astype(BF16_NP)
        ks = np.ascontiguousarray(wk_f[hs].T).reshape(C, P, HPC * HD).astype(BF16_NP)
        vs = np.ascontiguousarray(wv_f[hs].T).reshape(C, P, HPC * HD).astype(BF16_NP)
        # xres[g, p, cc, t] = x[token(g) + t, cc*128 + p]
        xr = np.empty((G, P, C, SL), dtype=np.float32)
        for g, tok0 in enumerate(_core_tokens(c, S)):
            xr[g] = xf[tok0 : tok0 + SL, :].T.reshape(C, P, SL).transpose(1, 0, 2)
        m = {
            "xbh": xbh,
            "xres": xr,
            "wq": qs,
            "wk": ks,
            "wv": vs,
            "woh": wohost,
            "wo3h": wo3host,
            "w1h": w1host,
            "w2h": w2host,
        }
        if len(mtiles_np):
            m["mk"] = np.stack(mtiles_np).astype(BF16_NP)
        in_maps.append(m)

    trace = os.environ.get("KTRACE", "0") == "1"
    res = run_bass_kernel_spmd(nc, in_maps, list(range(N_CORES)), trace=trace)
    global LAST_RESULT
    LAST_RESULT = res

    full = np.empty((B * S, D), dtype=np.float32)
    for core in range(N_CORES):
        o = res.results[core]["out"].reshape(D, QW)
        for g, tok0 in enumerate(_core_tokens(core, S)):
            full[tok0 : tok0 + SL, :] = o[:, ts(g, SL)].T
    return np.ascontiguousarray(full.reshape(B, S, D))


# revision 15
# speedup vs baseline: 1.0262x; 1.0262x over previous
"""Trainium2 Bass kernel for a dense transformer decoder block.

Strategy (8 NeuronCores):
  - Attention tensor-parallel over heads (2 heads/core); activations in
    transposed layout [D, tokens]; all matmuls bf16 with fp32 PSUM.
  - Instead of ReduceScattering the wo *outputs* (16.8 MB), the per-head
    attention outputs are exchanged with chunked AllToAlls (2 MB total,
    bf16, DeepSpeed-Ulysses style): after the A2A each core holds all 16
    heads for its own 512 tokens and computes the full wo locally (same
    FLOPs, overlapped chunk-by-chunk with the remaining attention), adds
    the exact f32 x residual from a per-core xres input, and keeps h in
    SBUF. No reduction collective at all.
  - FFN is data-parallel: each core runs the FULL FFN on its 512 tokens,
    streaming the full w1/w2 (bf16) from HBM under the matmuls. rmsnorm
    scaling is deferred through relu and the (linear) down-projection
    (relu(r*u) = r*relu(u), r>0), applied once on the 16 down outputs,
    so the PE never waits on the stats chain. Each core's (h + down) IS
    the final output for its tokens.
  - Engine balance: squares on DVE, exp on ACT, reciprocals via the fast
    DVE approximation, masks added via PE accumulation; stats computed
    one q-tile ahead so the PE runs dense.
  - Causality is not hardcoded: the mask input is classified host-side
    into skip / plain / mixed 128x512 blocks; mixed tiles are shipped
    as constants (4 distinct tiles for a causal mask).
"""

import os
import sys

try:  # the axon sitecustomize usually provides concourse already
    import concourse.bass  # noqa: F401
except ImportError:  # pragma: no cover
    sys.path.insert(0, "/opt/trn_rl_repo")

from contextlib import ExitStack

import ml_dtypes
import numpy as np

import concourse.bacc as bacc
import concourse.tile as tile
from concourse import mybir
from concourse.bass_utils import run_bass_kernel_spmd
from concourse.masks import make_identity

F32 = mybir.dt.float32
BF16 = mybir.dt.bfloat16
F16 = mybir.dt.float16
N_CORES = 8
P = 128
QW = 512  # q-tile / token-tile width
EPS = 1e-6
AF = mybir.ActivationFunctionType
ALU = mybir.AluOpType
BF16_NP = ml_dtypes.bfloat16
G = 4  # A2A chunks (one per pair of q-tiles)
SL = 128  # tokens per core-slice per chunk


def ts(i, w):
    return slice(i * w, (i + 1) * w)


def _classify_mask(mask, S):
    """mask: [S, S] additive (q, k). Returns (table, tiles).
    table[(kt, j)] = 'skip' | 'plain' | int mask-tile index.
    tiles: list of [128, QW] float32 arrays in scoresT ([k, q]) layout."""
    table = {}
    tiles = []
    keys = {}
    for j in range(S // QW):
        for kt in range(S // P):
            sub = mask[ts(j, QW), ts(kt, P)]  # [q, k]
            if np.all(sub <= -1e8):
                table[(kt, j)] = "skip"
            elif np.all(sub == 0.0):
                table[(kt, j)] = "plain"
            else:
                t = np.ascontiguousarray(sub.T.astype(np.float32))  # [k, q]
                key = t.tobytes()
                if key not in keys:
                    keys[key] = len(tiles)
                    tiles.append(t)
                table[(kt, j)] = keys[key]
    return table, tiles


def build_program(B, S, D, H, HID, mask_table, n_mask):
    HD = 128
    assert D == (D // P) * P and H * HD == D
    HPC = H // N_CORES            # heads per core
    assert HPC * N_CORES == H
    C = D // P                    # contraction chunks over D
    S_TILES = S // QW             # q tiles per batch
    KT = S // P                   # k tiles per batch
    HIDC = HID // P               # hidden tiles total (64)
    W1CH = 16                     # w1 stream chunks
    HTPC = HIDC // W1CH           # hid tiles per w1 chunk (4)

    nc = bacc.Bacc(trn_type="TRN2", num_devices=N_CORES)

    xbh = nc.dram_tensor(
        "xbh", [B, S // QW, P, D // P, QW], BF16, kind="ExternalInput"
    ).ap()
    xres = nc.dram_tensor("xres", [G, P, C, SL], F32, kind="ExternalInput").ap()
    wq = nc.dram_tensor("wq", [C, P, HPC * HD], BF16, kind="ExternalInput").ap()
    wk = nc.dram_tensor("wk", [C, P, HPC * HD], BF16, kind="ExternalInput").ap()
    wv = nc.dram_tensor("wv", [C, P, HPC * HD], BF16, kind="ExternalInput").ap()
    woh = nc.dram_tensor("woh", [P, H, D], BF16, kind="ExternalInput").ap()
    wo3h = nc.dram_tensor("wo3h", [C, P, H, P], BF16, kind="ExternalInput").ap()
    w1h = nc.dram_tensor("w1h", [W1CH, P, C, QW], BF16, kind="ExternalInput").ap()
    w2h = nc.dram_tensor("w2h", [C, P, HIDC, P], BF16, kind="ExternalInput").ap()
    mk = None
    if n_mask:
        mk = nc.dram_tensor("mk", [n_mask, P, QW], BF16, kind="ExternalInput").ap()

    a2a_in = [
        nc.dram_tensor(f"a2a_in{g}", [N_CORES, HPC, P, SL], BF16)
        for g in range(G)
    ]
    a2a_out = [
        nc.dram_tensor(f"a2a_out{g}", [N_CORES, HPC, P, SL], BF16)
        for g in range(G)
    ]
    out = nc.dram_tensor("out", [C, P, QW], F32, kind="ExternalOutput").ap()

    groups = [list(range(N_CORES))]

    with tile.TileContext(nc) as tc, ExitStack() as ctx:
        const = ctx.enter_context(tc.tile_pool(name="const", bufs=1))
        stats = ctx.enter_context(tc.tile_pool(name="stats", bufs=2))
        sqp = ctx.enter_context(tc.tile_pool(name="sq", bufs=2))
        hfp = ctx.enter_context(tc.tile_pool(name="hf", bufs=1))
        xrp = ctx.enter_context(tc.tile_pool(name="xr", bufs=1))
        aop = ctx.enter_context(tc.tile_pool(name="ao", bufs=2))
        psum = ctx.enter_context(tc.tile_pool(name="psum", bufs=1, space="PSUM"))

        ones_f32 = const.tile([P, P], F32)
        nc.vector.memset(ones_f32[:], 1.0)
        ones = const.tile([P, P], BF16)
        nc.vector.tensor_copy(ones[:], ones_f32[:])
        eps_p1 = const.tile([P, 1], F32)
        nc.vector.memset(eps_p1[:], EPS)
        ident = const.tile([P, P], F32)
        make_identity(nc, ident[:])
        ident_b = const.tile([P, P], BF16)
        nc.vector.tensor_copy(ident_b[:], ident[:])

        # h for this core's 512 tokens, assembled chunk by chunk
        hf = hfp.tile([P, C, QW], F16, tag="hf")

        # ---------------- attention phase ----------------
        with tc.tile_pool(name="wqkv", bufs=1) as wqkvp, \
             tc.tile_pool(name="xa", bufs=2) as xap, \
             tc.tile_pool(name="qkv", bufs=1) as qkvp, \
             tc.tile_pool(name="exp", bufs=3) as expp, \
             tc.tile_pool(name="attn", bufs=1) as attp:
            wq_sb = wqkvp.tile([P, C, HPC * HD], BF16, tag="wq")
            nc.sync.dma_start(wq_sb[:], wq.rearrange("c p o -> p c o"))
            wk_sb = wqkvp.tile([P, C, HPC * HD], BF16, tag="wk")
            nc.sync.dma_start(wk_sb[:], wk.rearrange("c p o -> p c o"))
            wv_sb = wqkvp.tile([P, C, HPC * HD], BF16, tag="wv")
            nc.scalar.dma_start(wv_sb[:], wv.rearrange("c p o -> p c o"))
            mtiles = None
            if n_mask:
                mtiles = wqkvp.tile([P, n_mask, QW], BF16, tag="mk")
                nc.scalar.dma_start(mtiles[:], mk.rearrange("n p q -> p n q"))
            wo_sb = wqkvp.tile([P, H, D], BF16, tag="wo")

            def emit_a2a(g):
                nc.gpsimd.collective_compute(
                    "AllToAll",
                    ALU.bypass,
                    replica_groups=groups,
                    ins=[a2a_in[g].ap().opt()],
                    outs=[a2a_out[g].ap().opt()],
                )

            def stats_mm(xb):
                """sq on DVE, column sums via PE."""
                cs = psum.tile([P, QW], F32, tag="mm", bufs=3)
                for c in range(C):
                    sq = sqp.tile([P, QW], BF16, tag="sq")
                    nc.vector.tensor_mul(sq[:], xb[:, c, :], xb[:, c, :])
                    nc.tensor.matmul(
                        cs[:], ones[:], sq[:], start=(c == 0), stop=(c == C - 1)
                    )
                return cs

            def stats_fin(cs):
                """sqrt on ACT (emitted outside the exp burst) + fast recip."""
                rms = stats.tile([P, QW], F32, tag="rms")
                nc.scalar.activation(
                    rms[:], cs[:], AF.Sqrt, bias=eps_p1[:], scale=1.0 / D
                )
                rinv = stats.tile([P, QW], F32, tag="rinv")
                nc.vector.reciprocal_approx_fast(rinv[:], rms[:])
                return rinv

            def emit_wo(g):
                """Local full-wo for this core's chunk-g tokens + x residual
                -> hf columns [g*SL, (g+1)*SL)."""
                ao = aop.tile([P, H, SL], BF16, tag="ao")
                nc.sync.dma_start(
                    ao[:], a2a_out[g].ap().rearrange("s h p t -> p (s h) t")
                )
                xr = xrp.tile([P, C, SL], F32, tag="xr")
                nc.gpsimd.dma_start(xr[:], xres[g])
                for ot in range(C):
                    po = psum.tile([P, SL], F32, tag="mm", bufs=3)
                    for oc in range(H):
                        nc.tensor.matmul(
                            po[:],
                            wo_sb[:, oc, ts(ot, P)],
                            ao[:, oc, :],
                            start=(oc == 0),
                            stop=(oc == H - 1),
                        )
                    nc.vector.tensor_add(
                        hf[:, ot, ts(g, SL)], xr[:, ot, :], po[:]
                    )

            for b in range(B):
                # prefetch x tiles of this batch (host-precast bf16; the
                # DMA hardware defers transfers until the WAR deps clear)
                xbs = []
                for j in range(S_TILES):
                    xb = xap.tile([P, C, QW], BF16, tag="xb", name=f"xb{b}_{j}")
                    nc.sync.dma_start(xb[:], xbh[b, j])
                    xbs.append(xb)

                rinv_next = stats_fin(stats_mm(xbs[0]))

                kT = qkvp.tile([P, HPC, S], BF16, tag="kT")
                vN = qkvp.tile([P, KT, HPC * HD], BF16, tag="vN")
                for j in range(S_TILES):
                    g = 2 * b + j // 2
                    r = j % 2
                    xb = xbs[j]
                    rinv = rinv_next
                    # q/k projections from RAW x; rinv folded in at eviction
                    qTs = qkvp.tile([P, HPC, QW], BF16, tag="qT", bufs=2)
                    for h in range(HPC):
                        for w_sb, dst in ((wq_sb, qTs), (wk_sb, kT)):
                            pp = psum.tile([P, QW], F32, tag="mm", bufs=3)
                            for c in range(C):
                                nc.tensor.matmul(
                                    pp[:],
                                    w_sb[:, c, ts(h, HD)],
                                    xb[:, c, :],
                                    start=(c == 0),
                                    stop=(c == C - 1),
                                )
                            if dst is qTs:
                                nc.vector.tensor_mul(qTs[:, h, :], pp[:], rinv[:])
                            else:
                                nc.vector.tensor_mul(
                                    kT[:, h, ts(j, QW)], pp[:], rinv[:]
                                )
                    # rinv transposed to token-partition layout for v scaling
                    rcol = stats.tile([P, QW // P], F32, tag="rcol")
                    for sub in range(QW // P):
                        tp = psum.tile([P, P], F32, tag="mm", bufs=3)
                        nc.tensor.transpose(tp[:], rinv[:, ts(sub, P)], ident[:])
                        nc.vector.tensor_copy(rcol[:, sub : sub + 1], tp[:, 0:1])
                    # v in natural layout; rinv via per-partition scalar
                    for sub in range(QW // P):
                        pv = psum.tile([P, QW], F32, tag="mm", bufs=3)
                        for c in range(C):
                            nc.tensor.matmul(
                                pv[:, : HPC * HD],
                                xb[:, c, ts(sub, P)],
                                wv_sb[:, c, :],
                                start=(c == 0),
                                stop=(c == C - 1),
                            )
                        nc.vector.tensor_scalar_mul(
                            vN[:, j * (QW // P) + sub, :],
                            pv[:, : HPC * HD],
                            rcol[:, sub : sub + 1],
                        )

                    # stats matmuls for the NEXT q-tile (ACT sqrt emitted
                    # after the exp burst below)
                    cs_next = stats_mm(xbs[j + 1]) if j + 1 < S_TILES else None

                    # -------- attention for q-tile j --------
                    attnT = attp.tile([P, HPC, QW], BF16, tag="attnT", bufs=2)
                    for h in range(HPC):
                        kts = [
                            kt for kt in range(KT) if mask_table[(kt, j)] != "skip"
                        ]
                        pa = psum.tile([P, QW], F32, tag="pv", bufs=1)
                        den = psum.tile([P, QW], F32, tag="stat", bufs=1)
                        n_k = len(kts)
                        exs = [None] * n_k

                        def _den_pv(i):
                            kt = kts[i]
                            nc.tensor.matmul(
                                den[:], ones[:], exs[i][:],
                                start=(i == 0), stop=(i == n_k - 1),
                            )
                            nc.tensor.matmul(
                                pa[:],
                                vN[:, kt, ts(h, HD)],
                                exs[i][:],
                                start=(i == 0),
                                stop=(i == n_k - 1),
                            )

                        for i, kt in enumerate(kts):
                            msc = psum.tile([P, QW], F32, tag="score", bufs=3)
                            mt = mask_table[(kt, j)]
                            if mt != "plain":
                                # additive mask via PE accumulation: PSUM
                                # starts as the mask, score accumulates on top
                                nc.tensor.matmul(
                                    msc[:], ident_b[:], mtiles[:, mt, :],
                                    start=True, stop=False,
                                )
                            nc.tensor.matmul(
                                msc[:],
                                kT[:, h, ts(kt, P)],
                                qTs[:, h, :],
                                start=(mt == "plain"),
                                stop=True,
                            )
                            ex = expp.tile([P, QW], BF16, tag="exp")
                            nc.scalar.activation(ex[:], msc[:], AF.Exp)
                            exs[i] = ex
                            if i > 0:
                                _den_pv(i - 1)
                        _den_pv(n_k - 1)
                        rec = stats.tile([P, QW], F32, tag="rec")
                        nc.vector.reciprocal_approx_fast(rec[:], den[:])
                        nc.vector.tensor_mul(attnT[:, h, :], pa[:], rec[:])

                    if cs_next is not None:
                        rinv_next = stats_fin(cs_next)

                    if r == 1 and g > 0:
                        # local wo for the PREVIOUS chunk, two q-tiles after
                        # its A2A was issued: robust to cross-core skew in
                        # the collective completion
                        emit_wo(g - 1)

                    # ---- ship attnT head-blocks into the A2A input ----
                    for h in range(HPC):
                        nc.sync.dma_start(
                            a2a_in[g].ap()[4 * r : 4 * r + 4, h, :, :].rearrange(
                                "s p t -> p s t"
                            ),
                            attnT[:, h, :],
                        )
                    if r == 1:
                        emit_a2a(g)
                        if g == 0:
                            # deferred big weight load: the gpsimd queue just
                            # unblocked at A2A-0 completion, so this transfer
                            # does not compete with the startup-critical DMAs
                            nc.gpsimd.dma_start(
                                wo_sb[:, :, : D // 2], woh[:, :, : D // 2]
                            )
                            nc.gpsimd.dma_start(
                                wo_sb[:, :, D // 2 :], woh[:, :, D // 2 :]
                            )
        # ---------------- FFN phase (data-parallel, 512 tokens/core) --------
        with tc.tile_pool(name="w1p", bufs=2) as w1p, \
             tc.tile_pool(name="w2p", bufs=2) as w2p, \
             tc.tile_pool(name="wo3p", bufs=2) as wo3p, \
             tc.tile_pool(name="up", bufs=1) as upp, \
             tc.tile_pool(name="oev", bufs=3) as oevp:
            up_sb = upp.tile([P, HIDC, QW], BF16, tag="up")
            HQ = QW // 2

            def up_pass(half):
                """up-projection for one half of the token columns; w1 is
                streamed (again) for each half. relu needs NO rinv (deferred
                through the down projection), so this runs with no stats dep:
                half A fills the A2A-3 + wo(3) tail with dense PE work."""
                cols = ts(half, HQ)
                for ch in range(W1CH):
                    w1c = w1p.tile([P, C, QW], BF16, tag="w1", name=f"w1{half}_{ch}")
                    nc.sync.dma_start(w1c[:], w1h[ch])
                    for hti in range(HTPC):
                        pu = psum.tile([P, HQ], F32, tag="mm", bufs=3)
                        for c in range(C):
                            nc.tensor.matmul(
                                pu[:],
                                w1c[:, c, ts(hti, P)],
                                hf[:, c, cols],
                                start=(c == 0),
                                stop=(c == C - 1),
                            )
                        nc.scalar.activation(
                            up_sb[:, ch * HTPC + hti, cols], pu[:], AF.Relu
                        )

            # ---- half A (token cols 0:256 = chunks 0,1) fills the tail ----
            up_pass(0)

            # ---- wo for the last chunk, with re-streamed wo weights ----
            ao = aop.tile([P, H, SL], BF16, tag="ao", name="ao3")
            nc.sync.dma_start(
                ao[:], a2a_out[G - 1].ap().rearrange("s h p t -> p (s h) t")
            )
            xr = xrp.tile([P, C, SL], F32, tag="xr", name="xr3")
            nc.sync.dma_start(xr[:], xres[G - 1])
            for ot in range(C):
                wo3 = wo3p.tile([P, H, P], BF16, tag="wo3")
                nc.sync.dma_start(wo3[:], wo3h[ot])
                po = psum.tile([P, SL], F32, tag="mm", bufs=3)
                for oc in range(H):
                    nc.tensor.matmul(
                        po[:],
                        wo3[:, oc, :],
                        ao[:, oc, :],
                        start=(oc == 0),
                        stop=(oc == H - 1),
                    )
                nc.vector.tensor_add(
                    hf[:, ot, ts(G - 1, SL)], xr[:, ot, :], po[:]
                )

            # rmsnorm2 stats (consumed only at the down outputs)
            cs = psum.tile([P, QW], F32, tag="mm", bufs=3)
            for c in range(C):
                sq = sqp.tile([P, QW], BF16, tag="sq")
                nc.vector.tensor_mul(sq[:], hf[:, c, :], hf[:, c, :])
                nc.tensor.matmul(
                    cs[:], ones[:], sq[:], start=(c == 0), stop=(c == C - 1)
                )
            rms2 = stats.tile([P, QW], F32, tag="rms")
            nc.scalar.activation(
                rms2[:], cs[:], AF.Sqrt, bias=eps_p1[:], scale=1.0 / D
            )
            r2 = stats.tile([P, QW], F32, tag="rinv")
            nc.vector.reciprocal_approx_fast(r2[:], rms2[:])

            # ---- half B (token cols 256:512) ----
            up_pass(1)

            for ot in range(C):
                w2c = w2p.tile([P, HIDC, P], BF16, tag="w2")
                nc.sync.dma_start(w2c[:], w2h[ot])
                pd = psum.tile([P, QW], F32, tag="mm", bufs=3)
                for hc in range(HIDC):
                    nc.tensor.matmul(
                        pd[:],
                        w2c[:, hc, :],
                        up_sb[:, hc, :],
                        start=(hc == 0),
                        stop=(hc == HIDC - 1),
                    )
                dn = oevp.tile([P, QW], F32, tag="dn")
                nc.vector.tensor_mul(dn[:], pd[:], r2[:])
                oev = oevp.tile([P, QW], F32, tag="oev")
                nc.vector.tensor_add(oev[:], hf[:, ot, :], dn[:])
                nc.sync.dma_start(out[ot], oev[:])

    nc.compile()
    return nc


_CACHE = {}
LAST_RESULT = None


def _get_program(B, S, D, H, HID, mask_table, n_mask, mask_key):
    key = (B, S, D, H, HID, mask_key)
    if key not in _CACHE:
        _CACHE[key] = build_program(B, S, D, H, HID, mask_table, n_mask)
    return _CACHE[key]


def _core_tokens(core, S):
    """Global token start for (core, chunk g) blocks of SL tokens."""
    toks = []
    for g in range(G):
        b = g // 2
        j = 2 * (g % 2) + core // 4
        toks.append(S * b + QW * j + SL * (core % 4))
    return toks


def kernel(x, mask, wq, wk, wv, wo, w1, w2, attn_norm_w, ffn_norm_w):
    x = np.asarray(x, dtype=np.float32)
    mask = np.asarray(mask, dtype=np.float32)
    wq, wk, wv, wo = (np.asarray(a, dtype=np.float32) for a in (wq, wk, wv, wo))
    w1, w2 = np.asarray(w1, dtype=np.float32), np.asarray(w2, dtype=np.float32)
    attn_norm_w = np.asarray(attn_norm_w, dtype=np.float32)
    ffn_norm_w = np.asarray(ffn_norm_w, dtype=np.float32)

    B, S, D = x.shape
    H = D // 128  # HD is fixed at 128 (= SBUF partition count)
    HID = w1.shape[0]
    HD = D // H
    HPC = H // N_CORES
    C = D // P
    HIDC = HID // P
    W1CH = 16

    mask_table, mtiles_np = _classify_mask(
        np.broadcast_to(mask, (1, 1, S, S))[0, 0], S
    )
    mask_key = hash(tuple(sorted((k, str(v)) for k, v in mask_table.items())))
    nc = _get_program(B, S, D, H, HID, mask_table, len(mtiles_np), mask_key)

    # ---- host-side prep ----
    # xbh[b, j, p, c, t] = x[b, j*QW + t, c*128 + p] in bf16
    xbh = np.ascontiguousarray(
        x.reshape(B, S // QW, QW, C, P).transpose(0, 1, 4, 3, 2)
    ).astype(BF16_NP)
    wq_f = (wq * attn_norm_w[None, :]) / np.sqrt(HD)
    wk_f = wk * attn_norm_w[None, :]
    wv_f = wv * attn_norm_w[None, :]
    w1_f = w1 * ffn_norm_w[None, :]

    # full weights, replicated on every core
    # woh[p, oc, o] = wo[o, oc*128 + p]
    wohost = np.ascontiguousarray(
        wo.T.reshape(H, P, D).transpose(1, 0, 2)
    ).astype(BF16_NP)
    # wo3h[ot, p, oc, o] = wo[ot*128 + o, oc*128 + p]
    wo3host = np.ascontiguousarray(
        wo.reshape(C, P, H, P).transpose(0, 3, 2, 1)
    ).astype(BF16_NP)
    # w1h[ch, p, c, o] = w1_f[hid = ch*512 + o, d = c*128 + p]
    w1host = np.ascontiguousarray(
        w1_f.reshape(W1CH, QW, C, P).transpose(0, 3, 2, 1)
    ).astype(BF16_NP)
    # w2h[ot, p, hc, o] = w2[d_out = ot*128 + o, hid = hc*128 + p]
    w2host = np.ascontiguousarray(
        w2.reshape(C, P, HIDC, P).transpose(0, 3, 2, 1)
    ).astype(BF16_NP)

    xf = x.reshape(B * S, D)
    in_maps = []
    for c in range(N_CORES):
        hs = slice(c * HPC * HD, (c + 1) * HPC * HD)
        qs = np.ascontiguousarray(wq_f[hs].T).reshape(C, P, HPC * HD).astype(BF16_NP)
        ks = np.ascontiguousarray(wk_f[hs].T).reshape(C, P, HPC * HD).astype(BF16_NP)
        vs = np.ascontiguousarray(wv_f[hs].T).reshape(C, P, HPC * HD).astype(BF16_NP)
        # xres[g, p, cc, t] = x[token(g) + t, cc*128 + p]
        xr = np.empty((G, P, C, SL), dtype=np.float32)
        for g, tok0 in enumerate(_core_tokens(c, S)):
            xr[g] = xf[tok0 : tok0 + SL, :].T.reshape(C, P, SL).transpose(1, 0, 2)
        m = {
            "xbh": xbh,
            "xres": xr,
            "wq": qs,
            "wk": ks,
            "wv": vs,
            "woh": wohost,
            "wo3h": wo3host,
            "w1h": w1host,
            "w2h": w2host,
        }
        if len(mtiles_np):
            m["mk"] = np.stack(mtiles_np).astype(BF16_NP)
        in_maps.append(m)

    trace = os.environ.get("KTRACE", "0") == "1"
    res = run_bass_kernel_spmd(nc, in_maps, list(range(N_CORES)), trace=trace)
    global LAST_RESULT
    LAST_RESULT = res

    full = np.empty((B * S, D), dtype=np.float32)
    for core in range(N_CORES):
        o = res.results[core]["out"].reshape(D, QW)
        for g, tok0 in enumerate(_core_tokens(core, S)):
            full[tok0 : tok0 + SL, :] = o[:, ts(g, SL)].T
    return np.ascontiguousarray(full.reshape(B, S, D))
